# revision 1
# baseline (speedup 1.0000x reference)
"""Trainium2 Bass kernel for the 2D stress-velocity acoustic FD propagator.

Strategy:
- 4 shots -> 4 NeuronCores, one full wavefield recurrence per core (SPMD,
  identical program; all per-shot variation moved into data).
- Grid [256,256] laid out as x -> (2 blocks x 128 partitions), z -> free dim.
- Per-core circular roll along x so the source sits at x'=0. The roll is
  exact: the zero rows/cols baked into the coefficient fields kill the
  seam coupling, and the x-difference operators are implemented as full
  circular shifts (PE shift-matmul + wrap-fix matmuls).
- Boundary zero-padding of the staggered differences is folded into the
  coefficient fields (bx/bz/kx/kz edges zeroed).
- z-diffs: shifted-AP tensor_tensor subs on DVE (guard columns).
- fac-multiplies on GPSIMD (Pool), combines on DVE, receiver column
  extraction on ACT, final gather on host.
"""

import numpy as np
import concourse.bass as bass
import concourse.bacc as bacc
import concourse.tile as tile
import concourse.mybir as mybir
from concourse.bass_utils import run_bass_kernel_spmd  # noqa: F401 (fallback path)

NGRID = 256
DXS = 10.0
DZS = 10.0
DTS = 1.0e-3
P = 128
W = 512  # data columns: 2 blocks x 256 z
GW = 520  # state tile width (guard col 0, data 1..512, guard 513+)

_PROG_CACHE = {}
LAST_EXEC_NS = None
LAST_RESULT = None


def _shift_mats():
    sf = np.zeros((128, 128), np.float32)  # dxf main: out[m]=p[m+1]-p[m]
    sf[np.arange(128), np.arange(128)] = -1.0
    sf[np.arange(1, 128), np.arange(127)] = 1.0
    cf = np.zeros((128, 128), np.float32)  # dxf wrap fix: out[127] += p[0 of other blk]
    cf[0, 127] = 1.0
    sb = np.zeros((128, 128), np.float32)  # dxb main: out[m]=v[m]-v[m-1]
    sb[np.arange(128), np.arange(128)] = 1.0
    sb[np.arange(127), np.arange(1, 128)] = -1.0
    cb = np.zeros((128, 128), np.float32)  # dxb wrap fix: out[0] -= v[127 of other blk]
    cb[127, 0] = -1.0
    return np.concatenate([sf, cf, sb, cb], axis=1)


import os

X_F32R = os.environ.get("X_F32R", "0") == "1"  # fast PE mode for shift matmuls


LOOPS = int(os.environ.get("KLOOPS", "1"))  # device-time benchmarking only


def _build(nt, sc, rc, ncores):
    f32 = mybir.dt.float32
    f32r = mybir.dt.float32r
    xdt = f32r if X_F32R else f32

    def xv(ap):  # view for x-diff matmul operands
        return ap.bitcast(xdt) if X_F32R else ap

    nc = bacc.Bacc(
        "TRN2", target_bir_lowering=False, debug=False, num_devices=ncores
    )
    coef_d = nc.dram_tensor("coef", [P, 5 * W], f32, kind="ExternalInput")
    mats_d = nc.dram_tensor("mats", [P, 4 * 128], f32, kind="ExternalInput")
    wav_d = nc.dram_tensor("wav", [1, nt], f32, kind="ExternalInput")
    rec_d = nc.dram_tensor("rec", [P, 2 * nt], f32, kind="ExternalOutput")

    with tile.TileContext(nc) as tc:
        with tc.tile_pool(name="const", bufs=1) as cp, \
             tc.tile_pool(name="state", bufs=1) as st, \
             tc.tile_pool(name="scr", bufs=3) as scr, \
             tc.tile_pool(name="ps", bufs=3, space="PSUM") as psp:
            coefT = cp.tile([P, 5 * W], f32)
            nc.sync.dma_start(coefT[:], coef_d[:])
            facT = coefT[:, 0:W]
            bxT = coefT[:, W:2 * W]
            bzT = coefT[:, 2 * W:3 * W]
            kxT = coefT[:, 3 * W:4 * W]
            kzT = coefT[:, 4 * W:5 * W]

            matsT = cp.tile([P, 4 * 128], f32)
            nc.sync.dma_start(matsT[:], mats_d[:])
            sfT = matsT[:, 0:128]
            cfT = matsT[:, 128:256]
            sbT = matsT[:, 256:384]
            cbT = matsT[:, 384:512]

            wavT = cp.tile([1, nt], f32)
            nc.sync.dma_start(wavT[:], wav_d[:])
            recT = cp.tile([P, 2 * nt], f32)

            pT = st.tile([P, GW], f32)
            vxT = st.tile([P, GW], f32)
            vzT = st.tile([P, GW], f32)
            nc.vector.memset(pT[:], 0.0)
            nc.vector.memset(vxT[:], 0.0)
            nc.vector.memset(vzT[:], 0.0)
            pD = pT[:, 1:1 + W]
            vxD = vxT[:, 1:1 + W]
            vzD = vzT[:, 1:1 + W]

            sfX, cfX, sbX, cbX = xv(sfT), xv(cfT), xv(sbT), xv(cbT)
            for tt in range(LOOPS * nt):
                t = tt % nt
                # off-chain fac multiplies (+source inject into fp) on Pool
                # (fvz first: its consumer vz runs earliest on DVE)
                fvz = scr.tile([P, W], f32, tag="fvz")
                nc.gpsimd.tensor_mul(fvz[:], facT, vzD)
                fvx = scr.tile([P, W], f32, tag="fvx")
                nc.gpsimd.tensor_mul(fvx[:], facT, vxD)
                fp = scr.tile([P, W], f32, tag="fp")
                nc.gpsimd.tensor_mul(fp[:], facT, pD)
                nc.gpsimd.tensor_add(fp[0:1, sc - 1:sc], fp[0:1, sc - 1:sc],
                                     wavT[0:1, t:t + 1])

                # dxf(p) circular on PE
                psA = psp.tile([P, W], f32, tag="psA")
                nc.tensor.matmul(psA[:], sfX, xv(pD), start=True, stop=False,
                                 skip_group_check=True)
                nc.tensor.matmul(psA[:, 0:256], cfX, xv(pT[:, 257:513]),
                                 start=False, stop=False, skip_group_check=True)
                nc.tensor.matmul(psA[:, 256:512], cfX, xv(pT[:, 1:257]),
                                 start=False, stop=True, skip_group_check=True)

                # full z-path on DVE: independent of psA, fills the PE window
                zf = scr.tile([P, W], f32, tag="zf")
                nc.vector.tensor_sub(zf[:], pT[:, 2:2 + W], pD)
                m1z = scr.tile([P, W], f32, tag="m1z")
                nc.vector.tensor_mul(m1z[:], bzT, zf[:])
                nc.gpsimd.tensor_sub(vzD, fvz[:], m1z[:])
                zb = scr.tile([P, W], f32, tag="zb")
                nc.vector.tensor_sub(zb[:], vzD, vzT[:, 0:W])

                # x-path
                m1x = scr.tile([P, W], f32, tag="m1x")
                nc.vector.tensor_mul(m1x[:], bxT, psA[:])
                nc.vector.tensor_sub(vxD, fvx[:], m1x[:])

                # dxb(vx) circular on PE
                psB = psp.tile([P, W], f32, tag="psB")
                nc.tensor.matmul(psB[:], sbX, xv(vxD), start=True, stop=False,
                                 skip_group_check=True)
                nc.tensor.matmul(psB[:, 0:256], cbX, xv(vxT[:, 257:513]),
                                 start=False, stop=False, skip_group_check=True)
                nc.tensor.matmul(psB[:, 256:512], cbX, xv(vxT[:, 1:257]),
                                 start=False, stop=True, skip_group_check=True)

                # combine: p = (fp+inj - m2z) - m2x ; m2z,t1 on Pool off-chain
                m2z = scr.tile([P, W], f32, tag="m2z")
                nc.gpsimd.tensor_mul(m2z[:], kzT, zb[:])
                t1 = scr.tile([P, W], f32, tag="t1")
                nc.gpsimd.tensor_sub(t1[:], fp[:], m2z[:])
                m2x = scr.tile([P, W], f32, tag="m2x")
                nc.vector.tensor_mul(m2x[:], kxT, psB[:])
                nc.vector.tensor_sub(pD, t1[:], m2x[:])

                # receiver columns (both blocks) on ACT
                nc.scalar.copy(recT[:, 2 * t:2 * t + 1], pT[:, rc:rc + 1])
                nc.scalar.copy(recT[:, 2 * t + 1:2 * t + 2],
                               pT[:, rc + 256:rc + 257])

            nc.sync.dma_start(rec_d[:], recT[:])
    nc.compile()
    return nc


class _Runner:
    """Builds the jax.jit(shard_map(bass_exec)) callable once; reusable."""

    def __init__(self, nc, ncores):
        import jax
        from jax.sharding import Mesh, PartitionSpec
        from jax.experimental.shard_map import shard_map
        from concourse import bass2jax

        bass2jax.install_neuronx_cc_hook()
        self.nc = nc
        self.ncores = ncores
        partition_name = (
            nc.partition_id_tensor.name if nc.partition_id_tensor else None
        )
        in_names, out_names, out_avals, zero_outs = [], [], [], []
        for alloc in nc.m.functions[0].allocations:
            if not isinstance(alloc, mybir.MemoryLocationSet):
                continue
            name = alloc.memorylocations[0].name
            if alloc.kind == "ExternalInput":
                if name != partition_name:
                    in_names.append(name)
            elif alloc.kind == "ExternalOutput":
                shape = tuple(alloc.tensor_shape)
                dtype = mybir.dt.np(alloc.dtype)
                out_names.append(name)
                out_avals.append(jax.core.ShapedArray(shape, dtype))
                zero_outs.append(np.zeros(shape, dtype))
        self.n_params = len(in_names)
        self.in_names = list(in_names)
        self.out_names = out_names
        self.zero_outs = zero_outs
        all_in_names = in_names + out_names
        if partition_name is not None:
            all_in_names.append(partition_name)

        def _body(*args):
            operands = list(args)
            if partition_name is not None:
                operands.append(bass2jax.partition_id_tensor())
            outs = bass2jax._bass_exec_p.bind(
                *operands,
                out_avals=tuple(out_avals),
                in_names=tuple(all_in_names),
                out_names=tuple(out_names),
                lowering_input_output_aliases=(),
                sim_require_finite=True,
                sim_require_nnan=True,
                nc=nc,
            )
            return tuple(outs)

        devices = jax.devices()[:ncores]
        self.mesh = Mesh(np.asarray(devices), ("core",))
        nio = self.n_params + len(out_names)
        self.fn = jax.jit(
            shard_map(
                _body,
                mesh=self.mesh,
                in_specs=(PartitionSpec("core"),) * nio,
                out_specs=(PartitionSpec("core"),) * len(out_names),
                check_rep=False,
            ),
            keep_unused=True,
        )

    def prep_args(self, in_maps):
        args = [
            np.concatenate([np.asarray(m[name]) for m in in_maps], axis=0)
            for name in self.in_names
        ]
        args += [
            np.concatenate([z] * self.ncores, axis=0) for z in self.zero_outs
        ]
        return args

    def __call__(self, in_maps):
        outs = self.fn(*self.prep_args(in_maps))
        results = []
        for c in range(self.ncores):
            d = {}
            for i, name in enumerate(self.out_names):
                rows = self.zero_outs[i].shape[0]
                d[name] = np.asarray(outs[i][c * rows:(c + 1) * rows])
            results.append(d)
        return results


def kernel(vp, rho, damp, wavelet, src_x, src_z, rcv_x, rcv_z):
    global LAST_EXEC_NS, LAST_RESULT
    vp = np.asarray(vp, dtype=np.float32)
    rho = np.asarray(rho, dtype=np.float32)
    damp = np.asarray(damp, dtype=np.float32)
    wavelet = np.asarray(wavelet, dtype=np.float32)
    src_x = np.asarray(src_x).astype(np.int64)
    src_z = np.asarray(src_z).astype(np.int64)
    rcv_x = np.asarray(rcv_x).astype(np.int64)
    rcv_z = np.asarray(rcv_z).astype(np.int64)

    ns, nt = wavelet.shape
    nx, nz = vp.shape
    assert nx == NGRID and nz == NGRID, (nx, nz)
    assert ns <= 8

    szs = set(int(v) for v in src_z)
    rzs = set(int(v) for v in rcv_z)
    assert len(szs) == 1, "kernel specialized for uniform src_z"
    assert len(rzs) == 1, "kernel specialized for uniform rcv_z"
    sz = szs.pop()
    rz = rzs.pop()
    sc = 1 + sz
    rc = 1 + rz

    f64 = np.float64
    kappa = rho.astype(f64) * vp.astype(f64) ** 2
    fac = (1.0 - DTS * damp.astype(f64)).astype(np.float32)
    bx = (DTS / DXS / rho.astype(f64)).astype(np.float32)
    bx[nx - 1, :] = 0.0
    bz = (DTS / DZS / rho.astype(f64)).astype(np.float32)
    bz[:, nz - 1] = 0.0
    kx = (DTS / DXS * kappa).astype(np.float32)
    kx[0, :] = 0.0
    kz = (DTS / DZS * kappa).astype(np.float32)
    kz[:, 0] = 0.0

    key = (nt, sc, rc, ns)
    if key not in _PROG_CACHE:
        _PROG_CACHE[key] = _Runner(_build(nt, sc, rc, ns), ns)
    runner = _PROG_CACHE[key]

    mats = _shift_mats()
    src_amp = DTS / (DXS * DZS)

    def fold(a):
        return np.ascontiguousarray(
            a.reshape(2, 128, 256).transpose(1, 0, 2).reshape(128, 512)
        )

    in_maps = []
    for s in range(ns):
        ox = int(src_x[s]) % nx

        def prep(a):
            return fold(np.roll(a, -ox, axis=0))

        coef = np.concatenate(
            [prep(fac), prep(bx), prep(bz), prep(kx), prep(kz)], axis=1
        )
        wav = (wavelet[s].astype(f64) * src_amp).astype(np.float32)[None, :]
        in_maps.append({"coef": coef, "mats": mats, "wav": wav})

    results = runner(in_maps)
    LAST_RESULT = results

    nr = len(rcv_x)
    out = np.zeros((ns, nt, nr), np.float32)
    for s in range(ns):
        cols = results[s]["rec"]  # [128, 2*nt] -> (part, (t, blk))
        flat = cols.reshape(128, nt, 2).transpose(1, 2, 0).reshape(nt, 256)
        xs = (rcv_x - int(src_x[s])) % nx
        out[s] = flat[:, xs]
    return out



# revision 9
# speedup vs baseline: 2.6829x; 2.6829x over previous
"""Trainium2 Bass kernel for the 2D stress-velocity acoustic FD propagator.

Strategy (v2, fp16):
- 4 shots -> 4 NeuronCores, one full wavefield recurrence per core (SPMD).
- Grid [256,256]: x -> (2 blocks x 128 partitions), z -> free dim.
- Per-core circular roll along x so the source sits at x'=0 (exact: zeroed
  coefficient rows kill the seam coupling).
- fp16 state + coefficients with velocity scaling alpha and source scaling
  sigma (linear system => output rescaled on host). DVE tensor_tensor runs
  in 2x mode on aligned fp16 operands.
- x-diffs: PE shift-matmuls (fp16, full rate). z-diffs: PE identity-matmul
  pairs into PSUM (avoids DVE 1x misaligned shifted-subs), staged to SBUF
  fp16 by the otherwise-idle ACT engine.
- Engine balance per step: DVE 8 ops, Pool 3 ops, PE 10 matmuls, ACT
  2 copies + source inject + receiver extraction.
"""

import numpy as np
import concourse.bass as bass
import concourse.bacc as bacc
import concourse.tile as tile
import concourse.mybir as mybir
from concourse.bass_utils import run_bass_kernel_spmd  # noqa: F401

NGRID = 256
DXS = 10.0
DZS = 10.0
DTS = 1.0e-3
P = 128
W = 512   # data columns: 2 blocks x 256 z
GW = 516  # state tile width: guards 0-1, data 2..513, guards 514-515
D0 = 2    # data start column (4-byte aligned for fp16 2x mode)

ALPHA = 3000.0  # velocity scaling (vx,vz stored *ALPHA)
SIGMA = 1.0e6   # source scaling (output /SIGMA on host)

_PROG_CACHE = {}
LAST_EXEC_NS = None
LAST_RESULT = None

import os

LOOPS = int(os.environ.get("KLOOPS", "1"))  # device-time benchmarking only


def _shift_mats():
    sf = np.zeros((128, 128), np.float16)  # dxf main: out[m]=p[m+1]-p[m]
    sf[np.arange(128), np.arange(128)] = -1.0
    sf[np.arange(1, 128), np.arange(127)] = 1.0
    cf = np.zeros((128, 128), np.float16)  # dxf wrap fix: out[127] += p[0 of other blk]
    cf[0, 127] = 1.0
    sb = np.zeros((128, 128), np.float16)  # dxb main: out[m]=v[m]-v[m-1]
    sb[np.arange(128), np.arange(128)] = 1.0
    sb[np.arange(127), np.arange(1, 128)] = -1.0
    cb = np.zeros((128, 128), np.float16)  # dxb wrap fix: out[0] -= v[127 of other blk]
    cb[127, 0] = -1.0
    return np.concatenate([sf, cf, sb, cb], axis=1)


def _window(t, sz):
    """Causal z-support of the wavefield at step t (margin 2/3)."""
    zlo = max(0, sz - t - 2)
    zhi = min(256, sz + t + 3)
    return zlo, zhi


def _build(nt, sz, rz, ncores):
    f32 = mybir.dt.float32
    f16 = mybir.dt.float16

    nc = bacc.Bacc(
        "TRN2", target_bir_lowering=False, debug=False, num_devices=ncores
    )
    coef_d = nc.dram_tensor("coef", [P, 5 * W], f16, kind="ExternalInput")
    mats_d = nc.dram_tensor("mats", [P, 4 * 128], f16, kind="ExternalInput")
    wav_d = nc.dram_tensor("wav", [1, nt], f16, kind="ExternalInput")
    rec_d = nc.dram_tensor("rec", [P, 2 * nt], f16, kind="ExternalOutput")

    with tile.TileContext(nc) as tc:
        with tc.tile_pool(name="const", bufs=1) as cp, \
             tc.tile_pool(name="state", bufs=1) as st, \
             tc.tile_pool(name="scr", bufs=3) as scr, \
             tc.tile_pool(name="ps", bufs=2, space="PSUM") as psp:
            coefT = cp.tile([P, 5 * W], f16)
            nc.sync.dma_start(coefT[:], coef_d[:])
            facT = coefT[:, 0:W]
            bxT = coefT[:, W:2 * W]
            bzT = coefT[:, 2 * W:3 * W]
            kxT = coefT[:, 3 * W:4 * W]
            kzT = coefT[:, 4 * W:5 * W]

            matsT = cp.tile([P, 4 * 128], f16)
            nc.sync.dma_start(matsT[:], mats_d[:])
            sfT = matsT[:, 0:128]
            cfT = matsT[:, 128:256]
            sbT = matsT[:, 256:384]
            cbT = matsT[:, 384:512]

            wavT = cp.tile([1, nt], f16)
            nc.sync.dma_start(wavT[:], wav_d[:])
            recT = cp.tile([P, 2 * nt], f16)

            pT = st.tile([P, GW], f16)
            vxT = st.tile([P, GW], f16)
            vzT = st.tile([P, GW], f16)
            nc.vector.memset(pT[:], 0.0)
            nc.vector.memset(vxT[:], 0.0)
            nc.vector.memset(vzT[:], 0.0)
            pD = pT[:, D0:D0 + W]
            vxD = vxT[:, D0:D0 + W]
            vzD = vzT[:, D0:D0 + W]

            # fac*state of the previous step, computed off-chain on Pool;
            # initialized to zero (state starts at zero).
            fvz = scr.tile([P, W], f16, tag="fvz0")
            nc.vector.memset(fvz[:], 0.0)
            fvx = scr.tile([P, W], f16, tag="fvx0")
            nc.vector.memset(fvx[:], 0.0)

            for tt in range(LOOPS * nt):
                t = tt % nt
                # causal z-window: step-t ops touch cols [a0,b0) (scratch)
                # / [a,b) (state, +D0). fac-muls prefetched for t+1 use w(t+1).
                zlo, zhi = _window(t, sz)
                a0, b0 = 2 * zlo, 2 * zhi
                a, b = D0 + a0, D0 + b0
                zlo1, zhi1 = _window(t + 1, sz)
                a01, b01 = 2 * zlo1, 2 * zhi1
                a1, b1 = D0 + a01, D0 + b01

                # ops reading the previous step's state come first
                fp = scr.tile([P, W], f16, tag="fp")
                nc.gpsimd.tensor_mul(fp[:, a0:b0], facT[:, a0:b0], pT[:, a:b])

                # dzf(p) on DVE: z-interleaved => shift is 2 cols (4B aligned)
                zf = scr.tile([P, W], f16, tag="zf")
                nc.vector.tensor_sub(zf[:, a0:b0], pT[:, a + 2:b + 2],
                                     pT[:, a:b])

                # dxf(p) circular on PE
                psA = psp.tile([P, W], f32, tag="psA")
                nc.tensor.matmul(psA[:, a0:b0], sfT, pT[:, a:b],
                                 start=True, stop=False, skip_group_check=True)
                nc.tensor.matmul(psA[:, a0:b0:2], cfT, pT[:, a + 1:b:2],
                                 start=False, stop=False, skip_group_check=True)
                nc.tensor.matmul(psA[:, a0 + 1:b0:2], cfT, pT[:, a:b:2],
                                 start=False, stop=True, skip_group_check=True)

                # source inject into fp on ACT (partition 0, block 0, z=sz)
                nc.scalar.activation(fp[0:1, 2 * sz:2 * sz + 1],
                                     fp[0:1, 2 * sz:2 * sz + 1],
                                     mybir.ActivationFunctionType.Identity,
                                     bias=wavT[0:1, t:t + 1], scale=1.0)

                # DVE queue in dependency-ready order
                m1z = scr.tile([P, W], f16, tag="m1z")
                nc.vector.tensor_mul(m1z[:, a0:b0], bzT[:, a0:b0], zf[:, a0:b0])
                nc.vector.tensor_sub(vzT[:, a:b], fvz[:, a0:b0], m1z[:, a0:b0])
                m1x = scr.tile([P, W], f16, tag="m1x")
                nc.vector.tensor_mul(m1x[:, a0:b0], bxT[:, a0:b0], psA[:, a0:b0])
                nc.vector.tensor_sub(vxT[:, a:b], fvx[:, a0:b0], m1x[:, a0:b0])

                # dzb(vz) on DVE
                zb = scr.tile([P, W], f16, tag="zb")
                nc.vector.tensor_sub(zb[:, a0:b0], vzT[:, a:b],
                                     vzT[:, a - 2:b - 2])

                # dxb(vx) circular on PE
                psB = psp.tile([P, W], f32, tag="psB")
                nc.tensor.matmul(psB[:, a0:b0], sbT, vxT[:, a:b],
                                 start=True, stop=False, skip_group_check=True)
                nc.tensor.matmul(psB[:, a0 + 1:b0:2], cbT, vxT[:, a:b:2],
                                 start=False, stop=False, skip_group_check=True)
                nc.tensor.matmul(psB[:, a0:b0:2], cbT, vxT[:, a + 1:b:2],
                                 start=False, stop=True, skip_group_check=True)

                # next step's fac*vz on Pool (off-chain, window t+1)
                fvz = scr.tile([P, W], f16, tag="fvz")
                nc.gpsimd.tensor_mul(fvz[:, a01:b01], facT[:, a01:b01],
                                     vzT[:, a1:b1])

                # combine on DVE
                m2z = scr.tile([P, W], f16, tag="m2z")
                nc.vector.tensor_mul(m2z[:, a0:b0], kzT[:, a0:b0], zb[:, a0:b0])
                t1 = scr.tile([P, W], f16, tag="t1")
                nc.vector.tensor_sub(t1[:, a0:b0], fp[:, a0:b0], m2z[:, a0:b0])
                m2x = scr.tile([P, W], f16, tag="m2x")
                nc.vector.tensor_mul(m2x[:, a0:b0], kxT[:, a0:b0], psB[:, a0:b0])
                nc.vector.tensor_sub(pT[:, a:b], t1[:, a0:b0], m2x[:, a0:b0])

                # next step's fac*vx on Pool (off-chain, window t+1)
                fvx = scr.tile([P, W], f16, tag="fvx")
                nc.gpsimd.tensor_mul(fvx[:, a01:b01], facT[:, a01:b01],
                                     vxT[:, a1:b1])

                # receiver columns: z-interleaved => contiguous pair
                nc.scalar.copy(recT[:, 2 * t:2 * t + 2],
                               pT[:, D0 + 2 * rz:D0 + 2 * rz + 2])

            nc.sync.dma_start(rec_d[:], recT[:])
    nc.compile()
    return nc


class _Runner:
    """Builds the jax.jit(shard_map(bass_exec)) callable once; reusable."""

    def __init__(self, nc, ncores):
        import jax
        from jax.sharding import Mesh, PartitionSpec
        from jax.experimental.shard_map import shard_map
        from concourse import bass2jax

        bass2jax.install_neuronx_cc_hook()
        self.nc = nc
        self.ncores = ncores
        partition_name = (
            nc.partition_id_tensor.name if nc.partition_id_tensor else None
        )
        in_names, out_names, out_avals, zero_outs = [], [], [], []
        for alloc in nc.m.functions[0].allocations:
            if not isinstance(alloc, mybir.MemoryLocationSet):
                continue
            name = alloc.memorylocations[0].name
            if alloc.kind == "ExternalInput":
                if name != partition_name:
                    in_names.append(name)
            elif alloc.kind == "ExternalOutput":
                shape = tuple(alloc.tensor_shape)
                dtype = mybir.dt.np(alloc.dtype)
                out_names.append(name)
                out_avals.append(jax.core.ShapedArray(shape, dtype))
                zero_outs.append(np.zeros(shape, dtype))
        self.n_params = len(in_names)
        self.in_names = list(in_names)
        self.out_names = out_names
        self.zero_outs = zero_outs
        all_in_names = in_names + out_names
        if partition_name is not None:
            all_in_names.append(partition_name)

        def _body(*args):
            operands = list(args)
            if partition_name is not None:
                operands.append(bass2jax.partition_id_tensor())
            outs = bass2jax._bass_exec_p.bind(
                *operands,
                out_avals=tuple(out_avals),
                in_names=tuple(all_in_names),
                out_names=tuple(out_names),
                lowering_input_output_aliases=(),
                sim_require_finite=True,
                sim_require_nnan=True,
                nc=nc,
            )
            return tuple(outs)

        devices = jax.devices()[:ncores]
        self.mesh = Mesh(np.asarray(devices), ("core",))
        nio = self.n_params + len(out_names)
        self.fn = jax.jit(
            shard_map(
                _body,
                mesh=self.mesh,
                in_specs=(PartitionSpec("core"),) * nio,
                out_specs=(PartitionSpec("core"),) * len(out_names),
                check_rep=False,
            ),
            keep_unused=True,
        )

    def prep_args(self, in_maps):
        args = [
            np.concatenate([np.asarray(m[name]) for m in in_maps], axis=0)
            for name in self.in_names
        ]
        args += [
            np.concatenate([z] * self.ncores, axis=0) for z in self.zero_outs
        ]
        return args

    def __call__(self, in_maps):
        outs = self.fn(*self.prep_args(in_maps))
        results = []
        for c in range(self.ncores):
            d = {}
            for i, name in enumerate(self.out_names):
                rows = self.zero_outs[i].shape[0]
                d[name] = np.asarray(outs[i][c * rows:(c + 1) * rows])
            results.append(d)
        return results


def kernel(vp, rho, damp, wavelet, src_x, src_z, rcv_x, rcv_z):
    global LAST_EXEC_NS, LAST_RESULT
    vp = np.asarray(vp, dtype=np.float32)
    rho = np.asarray(rho, dtype=np.float32)
    damp = np.asarray(damp, dtype=np.float32)
    wavelet = np.asarray(wavelet, dtype=np.float32)
    src_x = np.asarray(src_x).astype(np.int64)
    src_z = np.asarray(src_z).astype(np.int64)
    rcv_x = np.asarray(rcv_x).astype(np.int64)
    rcv_z = np.asarray(rcv_z).astype(np.int64)

    ns, nt = wavelet.shape
    nx, nz = vp.shape
    assert nx == NGRID and nz == NGRID, (nx, nz)
    assert ns <= 8

    szs = set(int(v) for v in src_z)
    rzs = set(int(v) for v in rcv_z)
    assert len(szs) == 1, "kernel specialized for uniform src_z"
    assert len(rzs) == 1, "kernel specialized for uniform rcv_z"
    sz = szs.pop()
    rz = rzs.pop()

    f64 = np.float64
    kappa = rho.astype(f64) * vp.astype(f64) ** 2
    fac = (1.0 - DTS * damp.astype(f64)).astype(np.float16)
    bx = (ALPHA * DTS / DXS / rho.astype(f64)).astype(np.float16)
    bx[nx - 1, :] = 0.0
    bz = (ALPHA * DTS / DZS / rho.astype(f64)).astype(np.float16)
    bz[:, nz - 1] = 0.0
    kx = (DTS / DXS * kappa / ALPHA).astype(np.float16)
    kx[0, :] = 0.0
    kz = (DTS / DZS * kappa / ALPHA).astype(np.float16)
    kz[:, 0] = 0.0

    key = (nt, sz, rz, ns)
    if key not in _PROG_CACHE:
        _PROG_CACHE[key] = _Runner(_build(nt, sz, rz, ns), ns)
    runner = _PROG_CACHE[key]

    mats = _shift_mats()
    src_amp = DTS / (DXS * DZS) * SIGMA

    def fold(a):
        # z-interleaved: col = 2*z + block  (x = block*128 + partition)
        return np.ascontiguousarray(
            a.reshape(2, 128, 256).transpose(1, 2, 0).reshape(128, 512)
        )

    in_maps = []
    for s in range(ns):
        ox = int(src_x[s]) % nx

        def prep(a):
            return fold(np.roll(a, -ox, axis=0))

        coef = np.concatenate(
            [prep(fac), prep(bx), prep(bz), prep(kx), prep(kz)], axis=1
        )
        wav = (wavelet[s].astype(f64) * src_amp).astype(np.float16)[None, :]
        in_maps.append({"coef": coef, "mats": mats, "wav": wav})

    results = runner(in_maps)
    LAST_RESULT = results

    nr = len(rcv_x)
    out = np.zeros((ns, nt, nr), np.float32)
    for s in range(ns):
        cols = results[s]["rec"].astype(np.float32) / SIGMA
        flat = cols.reshape(128, nt, 2).transpose(1, 2, 0).reshape(nt, 256)
        xs = (rcv_x - int(src_x[s])) % nx
        out[s] = flat[:, xs]
    return out
